# revision 10
# baseline (speedup 1.0000x reference)
"""MixtureOfExpertsTreeEnsemble Trainium2 kernel (8-core SPMD, hybrid shard).

Math (per batch row b, tree t):
  g[b,n,t] = sigmoid(x[b] @ W[n,:,t] + bias[n,t])          63 internal nodes
  p[b,l,t] = prod of g / (1-g) along root->leaf path        64 leaves
  w[l,d,t] = leaf_weight[l,d,t] * softmax_t(gates[l,d,t])
  out[b,d] = sum_{l,t} p[b,l,t] * w[l,d,t]

Sharding: hybrid 4-way batch x 2-way tree (the tree axis is independent
until the final sum, per the tree-parallel decomposition): core c handles
batch rows [(c%4)*1024, ...) and trees [(c//4)*32, ...).  Host sums the
two tree-half partial outputs per batch slice (no on-device collectives).
This halves the replicated W / leaf-table DMA per core vs pure batch DP.

Device-side structure (per core, all engines balanced under the PE roof):
  * phase A (PE):   logits [b_tile(128), (node,tree)] bf16 matmuls into
    [128,1024] 2-bank PSUM tiles; sigmoid per tile on ACT
  * phase 0 (ACT):  exp(gates) FIRST on ACT (one table load each for Exp
    and Sigmoid, no switching back); softmax reduce via 4x stt halving
    adds on DVE; per-leaf normalize split DVE/Pool
  * phase B (DVE):  leaf path probabilities by level doubling in block
    layout; every op is scalar_tensor_tensor with dummy scalar (4x mode)
  * phase C (DMA):  p and w transposed by DMA-transpose (xbar), entirely
    off the PE/DVE; no PSUM->SBUF copy needed
  * phase D (PE):   out_T[d,b] += wT_chunk.T @ pT_chunk per b-tile
  * DMA rings: W on SP, x/leaf tables/transposes/output on ACT
"""

import sys

sys.path.insert(0, "/opt/trn_rl_repo")

import ml_dtypes
import numpy as np

BF16 = np.dtype(ml_dtypes.bfloat16)

MAX_DEPTH = 6
NUM_TREES = 64
LEAF_DIMS = 128
D_IN = 512
BATCH = 4096
N_INTERNAL = 63
N_LEAVES = 64
N_CORES = 8

BSHARD = 4                     # batch-parallel ways
TSHARD = 2                     # tree-parallel ways
BS = BATCH // BSHARD           # 1024 batch rows per core
T = NUM_TREES // TSHARD        # 32 trees per core
KT = D_IN // 128               # 4 contraction tiles
NT = N_INTERNAL * T            # 2016 (node,tree) pairs
LT = N_LEAVES * T              # 2048 (leaf,tree) pairs
NBT = BS // 128                # 8 batch tiles per core
NPT = LT // 128                # 16 contraction chunks for phase D


def _bitrev(x: int, bits: int) -> int:
    r = 0
    for _ in range(bits):
        r = (r << 1) | (x & 1)
        x >>= 1
    return r


# block-recursion orderings (see phase B)
_NODES_PERM = np.array(
    [(2**lvl - 1) + _bitrev(j, lvl) for lvl in range(MAX_DEPTH) for j in range(2**lvl)]
)
_LEAF_PERM = np.array([_bitrev(j, MAX_DEPTH) for j in range(N_LEAVES)])

_BUILT = {}


def _build(use_bias: bool):
    """Build + finalize the per-core Bass program."""
    import concourse.bacc as bacc
    import concourse.tile as tile
    from concourse import mybir
    from concourse.masks import make_identity

    f32 = mybir.dt.float32
    bf16 = mybir.dt.bfloat16
    AF = mybir.ActivationFunctionType
    MUL = mybir.AluOpType.mult
    ADD = mybir.AluOpType.add
    SUB = mybir.AluOpType.subtract

    nc = bacc.Bacc("TRN2", target_bir_lowering=False, debug=False)

    xT = nc.dram_tensor("xT", [KT, 128, BS], bf16, kind="ExternalInput")
    Wf = nc.dram_tensor("Wf", [KT, 128, NT], bf16, kind="ExternalInput")
    # leaf tables host-transposed to [d, (leaf, tree)] for contiguous DMA.
    # gates carry ALL 64 trees (softmax normalizes across the full tree
    # axis), host-rolled so this core's 32 trees sit at t=0:32.
    gt = nc.dram_tensor("gt", [LEAF_DIMS, N_LEAVES * NUM_TREES], bf16,
                        kind="ExternalInput")
    lwt = nc.dram_tensor("lwt", [LEAF_DIMS, LT], bf16, kind="ExternalInput")
    if use_bias:
        bias = nc.dram_tensor("bias", [1, NT], bf16, kind="ExternalInput")
    outT = nc.dram_tensor("outT", [LEAF_DIMS, BS], f32, kind="ExternalOutput")

    # phase A chunking: per btile two [128,1024] PSUM tiles, each covering
    # two single-bank matmul groups
    HALves = [
        (0, [(0, 512), (512, 1024)]),
        (1024, [(1024, 1536), (1536, NT)]),
    ]

    with tile.TileContext(nc) as tc:
        with tc.tile_pool(name="const", bufs=1) as cpool, \
             tc.tile_pool(name="wts", bufs=1) as wpool, \
             tc.tile_pool(name="psA", bufs=3, space="PSUM") as psA, \
             tc.tile_pool(name="psO", bufs=1, space="PSUM") as psO:

            # ---- input DMAs.  x + leaf tables on the ACT HW-DGE ring, the
            # weight matrix on the SP ring, so the early pieces overlap. ----
            xk = wpool.tile([128, KT, BS], bf16, tag="xk")
            nc.scalar.dma_start(
                xk[:, :, 0:256], xT[:, :, 0:256].rearrange("k p b -> p k b"))
            gtile = wpool.tile([128, N_LEAVES, NUM_TREES], bf16, tag="gtile")
            nc.scalar.dma_start(
                gtile[:], gt[:, :].rearrange("d (l t) -> d l t", t=NUM_TREES))
            nc.scalar.dma_start(
                xk[:, :, 256:BS], xT[:, :, 256:BS].rearrange("k p b -> p k b"))
            lwtile = wpool.tile([128, N_LEAVES, T], bf16, tag="lwtile")
            nc.scalar.dma_start(
                lwtile[:], lwt[:, :].rearrange("d (l t) -> d l t", t=T))

            wk = wpool.tile([128, KT, NT], bf16, tag="wk")
            for k in range(KT):
                nc.sync.dma_start(wk[:, k, 0:512], Wf[k, :, 0:512])
            for k in range(KT):
                nc.sync.dma_start(wk[:, k, 512:1024], Wf[k, :, 512:1024])
            for k in range(KT):
                nc.sync.dma_start(wk[:, k, 1024:NT], Wf[k, :, 1024:NT])
            if use_bias:
                bias_sb = cpool.tile([1, NT], bf16, tag="bias")
                nc.sync.dma_start(bias_sb[:], bias[:, :])
                ones1 = cpool.tile([1, 128], bf16, tag="ones1")
                nc.gpsimd.memset(ones1[:], 1.0)

            ones32 = cpool.tile([128, 32], bf16, tag="ones32")
            nc.gpsimd.memset(ones32[:], 1.0)

            # ---- ACT queue head: exp of the gate table (its own act-func
            # table; loaded once, before the sigmoid table) ----
            nc.scalar.activation(gtile[:, 0:32, :], gtile[:, 0:32, :], AF.Exp)
            nc.scalar.activation(gtile[:, 32:64, :], gtile[:, 32:64, :], AF.Exp)
            # (each exp instr covers [128, 2048]; both run before any sigmoid
            # so Exp and Sigmoid tables each load exactly once)

            with tc.tile_pool(name="gp", bufs=1) as gpool, \
                 tc.tile_pool(name="pp", bufs=2) as ppool, \
                 tc.tile_pool(name="pfp", bufs=2) as pfpool, \
                 tc.tile_pool(name="ptp", bufs=1) as pTpool, \
                 tc.tile_pool(name="outp", bufs=1) as outpool:

                out_ps = [psO.tile([LEAF_DIMS, 512], f32, tag=f"out_ps{h}",
                                   name=f"out_ps{h}")
                          for h in range(2)]
                out_sb = outpool.tile([LEAF_DIMS, BS], f32, tag="out_sb")

                # PE warm-up: dummy transposes release the HAM clock gate
                # while the first weight DMAs are in flight
                ident32 = cpool.tile([128, 128], f32, tag="ident32")
                make_identity(nc, ident32[:])
                for _wi in range(10):
                    nc.tensor.transpose(out_ps[0][:, 0:128], ident32[:], ident32[:])

                pT_all = pTpool.tile([128, NPT, BS], bf16, tag="pT")
                wT_all = wpool.tile([128, NPT, 128], bf16, tag="wT")
                wsm = wpool.tile([128, N_LEAVES, T], bf16, tag="wsm")
                g_all = gpool.tile([128, NBT, NT], bf16, tag="g")

                def emit_phase0_reduce():
                    # softmax denominator over ALL 64 trees: halving adds
                    # (4x stt), then reciprocal
                    s32 = cpool.tile([128, N_LEAVES, 32], bf16, tag="s32")
                    nc.vector.scalar_tensor_tensor(
                        s32[:], gtile[:, :, 0:32], 1.0, gtile[:, :, 32:64],
                        op0=MUL, op1=ADD)
                    s16 = cpool.tile([128, N_LEAVES, 16], bf16, tag="s16")
                    nc.vector.scalar_tensor_tensor(
                        s16[:], s32[:, :, 0:16], 1.0, s32[:, :, 16:32],
                        op0=MUL, op1=ADD)
                    s8 = cpool.tile([128, N_LEAVES, 8], bf16, tag="s8")
                    nc.vector.scalar_tensor_tensor(
                        s8[:], s16[:, :, 0:8], 1.0, s16[:, :, 8:16],
                        op0=MUL, op1=ADD)
                    s4 = cpool.tile([128, N_LEAVES, 4], bf16, tag="s4")
                    nc.vector.scalar_tensor_tensor(
                        s4[:], s8[:, :, 0:4], 1.0, s8[:, :, 4:8],
                        op0=MUL, op1=ADD)
                    s2 = cpool.tile([128, N_LEAVES, 2], bf16, tag="s2")
                    nc.vector.scalar_tensor_tensor(
                        s2[:], s4[:, :, 0:2], 1.0, s4[:, :, 2:4],
                        op0=MUL, op1=ADD)
                    s1 = cpool.tile([128, N_LEAVES], f32, tag="s1")
                    nc.vector.scalar_tensor_tensor(
                        s1[:], s2[:, :, 0], 1.0, s2[:, :, 1],
                        op0=MUL, op1=ADD)
                    r = cpool.tile([128, N_LEAVES], f32, tag="r")
                    nc.vector.reciprocal(r[:], s1[:])
                    return r

                def emit_phase0_normalize(r):
                    # w = e * r * leaf_weight; per-leaf scalar, split DVE/Pool
                    for l in range(N_LEAVES):
                        nc.vector.scalar_tensor_tensor(
                            wsm[:, l, :], gtile[:, l, 0:T], r[:, l:l + 1],
                            lwtile[:, l, :], op0=MUL, op1=MUL)
                    # w transposed via DMA xbar on the (idle) SP ring
                    nc.sync.dma_start_transpose(wT_all[:], wsm[:])

                def emit_phaseD(i):
                    bsl = slice(i * 128, (i + 1) * 128)
                    ps = out_ps[i // 4]
                    osl = slice((i % 4) * 128, (i % 4 + 1) * 128)
                    for j in range(NPT):
                        nc.tensor.matmul(ps[:, osl],
                                         wT_all[:, j, :],
                                         pT_all[:, j, bsl],
                                         start=(j == 0), stop=(j == NPT - 1))
                    nc.vector.tensor_copy(out_sb[:, bsl], ps[:, osl])
                    nc.scalar.dma_start(outT[:, bsl], out_sb[:, bsl])

                for i in range(NBT):
                    bsl = slice(i * 128, (i + 1) * 128)
                    # ---- phase A ----
                    for off, chunks in HALves:
                        w_tot = chunks[-1][1] - off
                        lg = psA.tile([128, 1024], f32, tag="lg")
                        for c0, c1 in chunks:
                            r0, r1 = c0 - off, c1 - off
                            for k in range(KT):
                                nc.tensor.matmul(
                                    lg[:, r0:r1], xk[:, k, bsl], wk[:, k, c0:c1],
                                    start=(k == 0),
                                    stop=(k == KT - 1 and not use_bias),
                                )
                            if use_bias:
                                nc.tensor.matmul(
                                    lg[:, r0:r1], ones1[:], bias_sb[:, c0:c1],
                                    start=False, stop=True,
                                )
                        nc.scalar.activation(
                            g_all[:, i, off:off + w_tot], lg[:, 0:w_tot],
                            AF.Sigmoid)

                    # ---- phase B: level doubling, all 4x stt on DVE ----
                    g = g_all[:, i, :]
                    pa = ppool.tile([128, 1024], bf16, tag="pa")
                    pb = ppool.tile([128, 1024], bf16, tag="pb")
                    pf = pfpool.tile([128, LT], bf16, tag="pf")
                    nc.vector.tensor_copy(pa[:, 0:T], g[:, 0:T])
                    nc.vector.scalar_tensor_tensor(
                        pa[:, T:2 * T], g[:, 0:T], -1.0, ones32[:],
                        op0=MUL, op1=ADD)
                    cur = pa
                    for lvl in range(1, MAX_DEPTH):
                        h = (2 ** lvl) * T
                        off = (2 ** lvl - 1) * T
                        dst = pf if lvl == MAX_DEPTH - 1 else \
                            (pb if cur is pa else pa)
                        nc.vector.scalar_tensor_tensor(
                            dst[:, 0:h], cur[:, 0:h], 1.0, g[:, off:off + h],
                            op0=MUL, op1=MUL)
                        nc.vector.scalar_tensor_tensor(
                            dst[:, h:2 * h], cur[:, 0:h], 1.0, dst[:, 0:h],
                            op0=MUL, op1=SUB)
                        cur = dst

                    # ---- phase C: p transposed by the DMA xbar (ACT ring) ----
                    nc.scalar.dma_start_transpose(
                        pT_all[:, :, bsl], pf[:, :])

                    if i == 0:
                        r = emit_phase0_reduce()
                    if i == 1:
                        emit_phase0_normalize(r)
                    # phase D two btiles behind, so the PE queue never blocks
                    # on a pending transpose while phase-A work is ready
                    if i >= 2:
                        emit_phaseD(i - 2)
                emit_phaseD(NBT - 2)
                emit_phaseD(NBT - 1)

    nc.finalize()
    return nc


def _get_nc(use_bias: bool):
    if use_bias not in _BUILT:
        _BUILT[use_bias] = _build(use_bias)
    return _BUILT[use_bias]


def _make_in_maps(x, W, b, leaf_weight, gates):
    x = np.ascontiguousarray(np.asarray(x, dtype=np.float32))
    W = np.asarray(W, dtype=np.float32)
    b = np.asarray(b, dtype=np.float32)
    leaf_weight = np.asarray(leaf_weight, dtype=np.float32)
    gates = np.asarray(gates, dtype=np.float32)

    use_bias = bool(np.any(b))
    Wp = W[_NODES_PERM]                                   # [63, 512, 64]
    gp = gates[_LEAF_PERM]                                # [64, 128, 64]
    lp = leaf_weight[_LEAF_PERM]

    in_maps = []
    for c in range(N_CORES):
        bs = c % BSHARD
        th = c // BSHARD
        ts = slice(th * T, (th + 1) * T)
        troll = np.roll(np.arange(NUM_TREES), -th * T)    # own half first
        xs = x[bs * BS:(bs + 1) * BS]                     # [1024, 512]
        xTc = np.ascontiguousarray(xs.T.reshape(KT, 128, BS).astype(BF16))
        Wfc = np.ascontiguousarray(
            Wp[:, :, ts].transpose(1, 0, 2).reshape(KT, 128, NT).astype(BF16))
        gtc = np.ascontiguousarray(
            gp[:, :, troll].transpose(1, 0, 2).reshape(LEAF_DIMS, -1).astype(BF16))
        lwc = np.ascontiguousarray(
            lp[:, :, ts].transpose(1, 0, 2).reshape(LEAF_DIMS, LT).astype(BF16))
        m = {"xT": xTc, "Wf": Wfc, "gt": gtc, "lwt": lwc}
        if use_bias:
            m["bias"] = np.ascontiguousarray(
                b[_NODES_PERM][:, ts].reshape(1, NT).astype(BF16))
        in_maps.append(m)
    return use_bias, in_maps


def kernel(x, W, b, leaf_weight, gates):
    from concourse.bass_utils import run_bass_kernel_spmd

    use_bias, in_maps = _make_in_maps(x, W, b, leaf_weight, gates)
    nc = _get_nc(use_bias)

    res = run_bass_kernel_spmd(nc, in_maps, core_ids=list(range(N_CORES)))
    out = np.empty((BATCH, LEAF_DIMS), dtype=np.float32)
    for c in range(BSHARD):
        out[c * BS:(c + 1) * BS] = (
            res.results[c]["outT"] + res.results[c + BSHARD]["outT"]).T
    return out


# revision 12
# speedup vs baseline: 1.1242x; 1.1242x over previous
"""MixtureOfExpertsTreeEnsemble Trainium2 kernel (8-core SPMD, hybrid shard).

Math (per batch row b, tree t):
  g[b,n,t] = sigmoid(x[b] @ W[n,:,t] + bias[n,t])          63 internal nodes
  p[b,l,t] = prod of g / (1-g) along root->leaf path        64 leaves
  w[l,d,t] = leaf_weight[l,d,t] * softmax_t(gates[l,d,t])
  out[b,d] = sum_{l,t} p[b,l,t] * w[l,d,t]

Sharding: hybrid 4-way batch x 2-way tree (the tree axis is independent
until the final sum, per the tree-parallel decomposition): core c handles
batch rows [(c%4)*1024, ...) and trees [(c//4)*32, ...).  Host sums the
two tree-half partial outputs per batch slice (no on-device collectives).
This halves the replicated W / leaf-table DMA per core vs pure batch DP.

Device-side structure (per core, all engines balanced under the PE roof):
  * phase A (PE):   logits [b_tile(128), (node,tree)] bf16 matmuls into
    [128,1024] 2-bank PSUM tiles; sigmoid per tile on ACT
  * phase 0 (ACT):  exp(gates) FIRST on ACT (one table load each for Exp
    and Sigmoid, no switching back); softmax reduce via 4x stt halving
    adds on DVE; per-leaf normalize split DVE/Pool
  * phase B (DVE):  leaf path probabilities by level doubling in block
    layout; every op is scalar_tensor_tensor with dummy scalar (4x mode)
  * phase C (DMA):  p and w transposed by DMA-transpose (xbar), entirely
    off the PE/DVE; no PSUM->SBUF copy needed
  * phase D (PE):   out_T[d,b] += wT_chunk.T @ pT_chunk per b-tile
  * DMA rings: W on SP, x/leaf tables/transposes/output on ACT
"""

import sys

sys.path.insert(0, "/opt/trn_rl_repo")

import ml_dtypes
import numpy as np

BF16 = np.dtype(ml_dtypes.bfloat16)

MAX_DEPTH = 6
NUM_TREES = 64
LEAF_DIMS = 128
D_IN = 512
BATCH = 4096
N_INTERNAL = 63
N_LEAVES = 64
N_CORES = 8

BSHARD = 4                     # batch-parallel ways
TSHARD = 2                     # tree-parallel ways
BS = BATCH // BSHARD           # 1024 batch rows per core
T = NUM_TREES // TSHARD        # 32 trees per core
KT = D_IN // 128               # 4 contraction tiles
NT = N_INTERNAL * T            # 2016 (node,tree) pairs
LT = N_LEAVES * T              # 2048 (leaf,tree) pairs
NBT = BS // 128                # 8 batch tiles per core
NPT = LT // 128                # 16 contraction chunks for phase D


def _bitrev(x: int, bits: int) -> int:
    r = 0
    for _ in range(bits):
        r = (r << 1) | (x & 1)
        x >>= 1
    return r


# block-recursion orderings (see phase B)
_NODES_PERM = np.array(
    [(2**lvl - 1) + _bitrev(j, lvl) for lvl in range(MAX_DEPTH) for j in range(2**lvl)]
)
_LEAF_PERM = np.array([_bitrev(j, MAX_DEPTH) for j in range(N_LEAVES)])

_BUILT = {}


def _build(use_bias: bool):
    """Build + finalize the per-core Bass program."""
    import concourse.bacc as bacc
    import concourse.tile as tile
    from concourse import mybir
    from concourse.masks import make_identity

    f32 = mybir.dt.float32
    bf16 = mybir.dt.bfloat16
    AF = mybir.ActivationFunctionType
    MUL = mybir.AluOpType.mult
    ADD = mybir.AluOpType.add
    SUB = mybir.AluOpType.subtract

    nc = bacc.Bacc("TRN2", target_bir_lowering=False, debug=False)

    xT = nc.dram_tensor("xT", [KT, 128, BS], bf16, kind="ExternalInput")
    Wf = nc.dram_tensor("Wf", [KT, 128, NT], bf16, kind="ExternalInput")
    # leaf tables host-transposed to [d, (leaf, tree)] for contiguous DMA.
    # gates carry ALL 64 trees (softmax normalizes across the full tree
    # axis), host-rolled so this core's 32 trees sit at t=0:32.
    gt = nc.dram_tensor("gt", [LEAF_DIMS, N_LEAVES * NUM_TREES], bf16,
                        kind="ExternalInput")
    lwt = nc.dram_tensor("lwt", [LEAF_DIMS, LT], bf16, kind="ExternalInput")
    if use_bias:
        bias = nc.dram_tensor("bias", [1, NT], bf16, kind="ExternalInput")
    outT = nc.dram_tensor("outT", [LEAF_DIMS, BS], f32, kind="ExternalOutput")

    # phase A chunking: per btile two [128,1024] PSUM tiles, each covering
    # two single-bank matmul groups
    HALves = [
        (0, [(0, 512), (512, 1024)]),
        (1024, [(1024, 1536), (1536, NT)]),
    ]

    with tile.TileContext(nc) as tc:
        with tc.tile_pool(name="const", bufs=1) as cpool, \
             tc.tile_pool(name="wts", bufs=1) as wpool, \
             tc.tile_pool(name="psA", bufs=3, space="PSUM") as psA, \
             tc.tile_pool(name="psO", bufs=1, space="PSUM") as psO:

            # ---- input DMAs.  x + leaf tables on the ACT HW-DGE ring, the
            # weight matrix on the SP ring, so the early pieces overlap. ----
            xk = wpool.tile([128, KT, BS], bf16, tag="xk")
            nc.scalar.dma_start(
                xk[:, :, 0:256], xT[:, :, 0:256].rearrange("k p b -> p k b"))
            gtile = wpool.tile([128, N_LEAVES, NUM_TREES], bf16, tag="gtile")
            nc.scalar.dma_start(
                gtile[:], gt[:, :].rearrange("d (l t) -> d l t", t=NUM_TREES))
            nc.scalar.dma_start(
                xk[:, :, 256:BS], xT[:, :, 256:BS].rearrange("k p b -> p k b"))
            lwtile = wpool.tile([128, N_LEAVES, T], bf16, tag="lwtile")
            nc.scalar.dma_start(
                lwtile[:], lwt[:, :].rearrange("d (l t) -> d l t", t=T))

            wk = wpool.tile([128, KT, NT], bf16, tag="wk")
            for k in range(KT):
                nc.sync.dma_start(wk[:, k, 0:512], Wf[k, :, 0:512])
            for k in range(KT):
                nc.sync.dma_start(wk[:, k, 512:1024], Wf[k, :, 512:1024])
            for k in range(KT):
                nc.sync.dma_start(wk[:, k, 1024:NT], Wf[k, :, 1024:NT])
            if use_bias:
                bias_sb = cpool.tile([1, NT], bf16, tag="bias")
                nc.sync.dma_start(bias_sb[:], bias[:, :])
                ones1 = cpool.tile([1, 128], bf16, tag="ones1")
                nc.gpsimd.memset(ones1[:], 1.0)

            ones32 = cpool.tile([128, 32], bf16, tag="ones32")
            nc.gpsimd.memset(ones32[:], 1.0)
            # stt scalars as real [128,1] SBUF APs: a float-constant scalar
            # materializes an f32 const with no bass_ap, which defeats the
            # DVE 4x perf-mode check; an AP with free_size==1 is exempt
            one_sc = cpool.tile([128, 1], bf16, tag="one_sc")
            nc.gpsimd.memset(one_sc[:], 1.0)
            mone_sc = cpool.tile([128, 1], bf16, tag="mone_sc")
            nc.gpsimd.memset(mone_sc[:], -1.0)

            # ---- ACT queue head: exp of the gate table (its own act-func
            # table; loaded once, before the sigmoid table) ----
            nc.scalar.activation(gtile[:, 0:32, :], gtile[:, 0:32, :], AF.Exp)
            nc.scalar.activation(gtile[:, 32:64, :], gtile[:, 32:64, :], AF.Exp)
            # (each exp instr covers [128, 2048]; both run before any sigmoid
            # so Exp and Sigmoid tables each load exactly once)

            with tc.tile_pool(name="gp", bufs=1) as gpool, \
                 tc.tile_pool(name="pp", bufs=2) as ppool, \
                 tc.tile_pool(name="pfp", bufs=2) as pfpool, \
                 tc.tile_pool(name="ptp", bufs=1) as pTpool, \
                 tc.tile_pool(name="outp", bufs=1) as outpool:

                out_ps = [psO.tile([LEAF_DIMS, 512], f32, tag=f"out_ps{h}",
                                   name=f"out_ps{h}")
                          for h in range(2)]
                out_sb = outpool.tile([LEAF_DIMS, BS], f32, tag="out_sb")

                # PE warm-up: dummy transposes release the HAM clock gate
                # while the first weight DMAs are in flight
                ident32 = cpool.tile([128, 128], f32, tag="ident32")
                make_identity(nc, ident32[:])
                for _wi in range(10):
                    nc.tensor.transpose(out_ps[0][:, 0:128], ident32[:], ident32[:])

                pT_all = pTpool.tile([128, NPT, BS], bf16, tag="pT")
                wT_all = wpool.tile([128, NPT, 128], bf16, tag="wT")
                wsm = wpool.tile([128, N_LEAVES, T], bf16, tag="wsm")
                g_all = gpool.tile([128, NBT, NT], bf16, tag="g")

                def stt(out, in0, sc, in1, o0, o1):
                    nc.vector.scalar_tensor_tensor(
                        out, in0, sc, in1, op0=o0, op1=o1)

                def emit_phase0_reduce():
                    # softmax denominator over ALL 64 trees: halving adds
                    # (4x stt), then reciprocal
                    s32 = cpool.tile([128, N_LEAVES, 32], bf16, tag="s32")
                    stt(s32[:], gtile[:, :, 0:32], one_sc[:],
                        gtile[:, :, 32:64], MUL, ADD)
                    s16 = cpool.tile([128, N_LEAVES, 16], bf16, tag="s16")
                    stt(s16[:], s32[:, :, 0:16], one_sc[:], s32[:, :, 16:32],
                        MUL, ADD)
                    s8 = cpool.tile([128, N_LEAVES, 8], bf16, tag="s8")
                    stt(s8[:], s16[:, :, 0:8], one_sc[:], s16[:, :, 8:16],
                        MUL, ADD)
                    s4 = cpool.tile([128, N_LEAVES, 4], bf16, tag="s4")
                    stt(s4[:], s8[:, :, 0:4], one_sc[:], s8[:, :, 4:8],
                        MUL, ADD)
                    s2 = cpool.tile([128, N_LEAVES, 2], bf16, tag="s2")
                    stt(s2[:], s4[:, :, 0:2], one_sc[:], s4[:, :, 2:4],
                        MUL, ADD)
                    s1 = cpool.tile([128, N_LEAVES], f32, tag="s1")
                    stt(s1[:], s2[:, :, 0], one_sc[:], s2[:, :, 1], MUL, ADD)
                    r = cpool.tile([128, N_LEAVES], f32, tag="r")
                    nc.vector.reciprocal(r[:], s1[:])
                    return r

                def emit_phase0_normalize(r):
                    # w = e * r * leaf_weight; per-leaf scalar (rank-2 (l,d)
                    # normalizer forces one op per leaf)
                    for l in range(N_LEAVES):
                        stt(wsm[:, l, :], gtile[:, l, 0:T], r[:, l:l + 1],
                            lwtile[:, l, :], MUL, MUL)
                    # w transposed via DMA xbar on the (now idle) SP ring
                    nc.sync.dma_start_transpose(wT_all[:], wsm[:])

                def emit_phaseD(i):
                    bsl = slice(i * 128, (i + 1) * 128)
                    ps = out_ps[i // 4]
                    osl = slice((i % 4) * 128, (i % 4 + 1) * 128)
                    for j in range(NPT):
                        nc.tensor.matmul(ps[:, osl],
                                         wT_all[:, j, :],
                                         pT_all[:, j, bsl],
                                         start=(j == 0), stop=(j == NPT - 1))

                def emit_phaseB_pair(i):
                    # level doubling for btiles (i, i+1) in one set of 4x stt
                    # ops (3D APs; pairing halves the per-op access overhead)
                    g = g_all[:, i:i + 2, :]
                    pa = ppool.tile([128, 2, 1024], bf16, tag="pa")
                    pb = ppool.tile([128, 2, 1024], bf16, tag="pb")
                    pf = pfpool.tile([128, 2, LT], bf16, tag="pf")
                    nc.vector.tensor_copy(pa[:, :, 0:T], g[:, :, 0:T])
                    stt(pa[:, :, T:2 * T], g[:, :, 0:T], mone_sc[:],
                        ones2[:], MUL, ADD)
                    cur = pa
                    for lvl in range(1, MAX_DEPTH):
                        h = (2 ** lvl) * T
                        off = (2 ** lvl - 1) * T
                        dst = pf if lvl == MAX_DEPTH - 1 else \
                            (pb if cur is pa else pa)
                        stt(dst[:, :, 0:h], cur[:, :, 0:h], one_sc[:],
                            g[:, :, off:off + h], MUL, MUL)
                        stt(dst[:, :, h:2 * h], cur[:, :, 0:h], one_sc[:],
                            dst[:, :, 0:h], MUL, SUB)
                        cur = dst
                    return pf

                ones2 = cpool.tile([128, 2, T], bf16, tag="ones2")
                nc.gpsimd.memset(ones2[:], 1.0)

                for i in range(NBT):
                    bsl = slice(i * 128, (i + 1) * 128)
                    # ---- phase A ----
                    for off, chunks in HALves:
                        w_tot = chunks[-1][1] - off
                        lg = psA.tile([128, 1024], f32, tag="lg")
                        for c0, c1 in chunks:
                            r0, r1 = c0 - off, c1 - off
                            for k in range(KT):
                                nc.tensor.matmul(
                                    lg[:, r0:r1], xk[:, k, bsl], wk[:, k, c0:c1],
                                    start=(k == 0),
                                    stop=(k == KT - 1 and not use_bias),
                                )
                            if use_bias:
                                nc.tensor.matmul(
                                    lg[:, r0:r1], ones1[:], bias_sb[:, c0:c1],
                                    start=False, stop=True,
                                )
                        nc.scalar.activation(
                            g_all[:, i, off:off + w_tot], lg[:, 0:w_tot],
                            AF.Sigmoid)

                    if i % 2 == 1:
                        # ---- phase B for (i-1, i), then phase C: both
                        # p transposes via DMA xbar on the SP ring (keeps
                        # the ACT sequencer free for sigmoids) ----
                        pf = emit_phaseB_pair(i - 1)
                        nc.sync.dma_start_transpose(
                            pT_all[:, :, (i - 1) * 128:i * 128], pf[:, 0, :])
                        nc.sync.dma_start_transpose(
                            pT_all[:, :, bsl], pf[:, 1, :])

                    if i == 1:
                        r = emit_phase0_reduce()
                    if i == 2:
                        emit_phase0_normalize(r)
                    # phase D trails so the PE queue never blocks on a
                    # pending transpose while phase-A work is ready
                    if i >= 3:
                        emit_phaseD(i - 3)
                for i in range(NBT - 3, NBT):
                    emit_phaseD(i)
                # output tail: PSUM -> SBUF -> DRAM (SP ring)
                for i in range(NBT):
                    bsl = slice(i * 128, (i + 1) * 128)
                    nc.vector.tensor_copy(
                        out_sb[:, bsl],
                        out_ps[i // 4][:, (i % 4) * 128:(i % 4 + 1) * 128])
                    nc.sync.dma_start(outT[:, bsl], out_sb[:, bsl])

    nc.finalize()
    return nc


def _get_nc(use_bias: bool):
    if use_bias not in _BUILT:
        _BUILT[use_bias] = _build(use_bias)
    return _BUILT[use_bias]


def _make_in_maps(x, W, b, leaf_weight, gates):
    x = np.ascontiguousarray(np.asarray(x, dtype=np.float32))
    W = np.asarray(W, dtype=np.float32)
    b = np.asarray(b, dtype=np.float32)
    leaf_weight = np.asarray(leaf_weight, dtype=np.float32)
    gates = np.asarray(gates, dtype=np.float32)

    use_bias = bool(np.any(b))
    Wp = W[_NODES_PERM]                                   # [63, 512, 64]
    gp = gates[_LEAF_PERM]                                # [64, 128, 64]
    lp = leaf_weight[_LEAF_PERM]

    in_maps = []
    for c in range(N_CORES):
        bs = c % BSHARD
        th = c // BSHARD
        ts = slice(th * T, (th + 1) * T)
        troll = np.roll(np.arange(NUM_TREES), -th * T)    # own half first
        xs = x[bs * BS:(bs + 1) * BS]                     # [1024, 512]
        xTc = np.ascontiguousarray(xs.T.reshape(KT, 128, BS).astype(BF16))
        Wfc = np.ascontiguousarray(
            Wp[:, :, ts].transpose(1, 0, 2).reshape(KT, 128, NT).astype(BF16))
        gtc = np.ascontiguousarray(
            gp[:, :, troll].transpose(1, 0, 2).reshape(LEAF_DIMS, -1).astype(BF16))
        lwc = np.ascontiguousarray(
            lp[:, :, ts].transpose(1, 0, 2).reshape(LEAF_DIMS, LT).astype(BF16))
        m = {"xT": xTc, "Wf": Wfc, "gt": gtc, "lwt": lwc}
        if use_bias:
            m["bias"] = np.ascontiguousarray(
                b[_NODES_PERM][:, ts].reshape(1, NT).astype(BF16))
        in_maps.append(m)
    return use_bias, in_maps


def kernel(x, W, b, leaf_weight, gates):
    from concourse.bass_utils import run_bass_kernel_spmd

    use_bias, in_maps = _make_in_maps(x, W, b, leaf_weight, gates)
    nc = _get_nc(use_bias)

    res = run_bass_kernel_spmd(nc, in_maps, core_ids=list(range(N_CORES)))
    out = np.empty((BATCH, LEAF_DIMS), dtype=np.float32)
    for c in range(BSHARD):
        out[c * BS:(c + 1) * BS] = (
            res.results[c]["outT"] + res.results[c + BSHARD]["outT"]).T
    return out


# revision 15
# speedup vs baseline: 1.3595x; 1.2092x over previous
"""MixtureOfExpertsTreeEnsemble Trainium2 kernel (8-core SPMD, hybrid shard).

Math (per batch row b, tree t):
  g[b,n,t] = sigmoid(x[b] @ W[n,:,t] + bias[n,t])          63 internal nodes
  p[b,l,t] = prod of g / (1-g) along root->leaf path        64 leaves
  w[l,d,t] = leaf_weight[l,d,t] * softmax_t(gates[l,d,t])
  out[b,d] = sum_{l,t} p[b,l,t] * w[l,d,t]

Sharding: hybrid 4-way batch x 2-way tree (the tree axis is independent
until the final sum, per the tree-parallel decomposition): core c handles
batch rows [(c%4)*1024, ...) and trees [(c//4)*32, ...).  Host sums the
two tree-half partial outputs per batch slice (no on-device collectives).
This halves the replicated W / leaf-table DMA per core vs pure batch DP.

Device-side structure (per core, all engines balanced under the PE roof):
  * phase A (PE):   logits [b_tile(128), (node,tree)] bf16 matmuls into
    [128,1024] 2-bank PSUM tiles; sigmoid per tile on ACT
  * phase 0 (ACT):  exp(gates) FIRST on ACT (one table load each for Exp
    and Sigmoid, no switching back); softmax reduce via 4x stt halving
    adds on DVE; per-leaf normalize split DVE/Pool
  * phase B (DVE):  leaf path probabilities by level doubling in block
    layout; every op is scalar_tensor_tensor with dummy scalar (4x mode)
  * phase C (DMA):  p and w transposed by DMA-transpose (xbar), entirely
    off the PE/DVE; no PSUM->SBUF copy needed
  * phase D (PE):   out_T[d,b] += wT_chunk.T @ pT_chunk per b-tile
  * DMA rings: W on SP, x/leaf tables/transposes/output on ACT
"""

import sys

sys.path.insert(0, "/opt/trn_rl_repo")

import ml_dtypes
import numpy as np

BF16 = np.dtype(ml_dtypes.bfloat16)

MAX_DEPTH = 6
NUM_TREES = 64
LEAF_DIMS = 128
D_IN = 512
BATCH = 4096
N_INTERNAL = 63
N_LEAVES = 64
N_CORES = 8

BSHARD = 4                     # batch-parallel ways
TSHARD = 2                     # tree-parallel ways
BS = BATCH // BSHARD           # 1024 batch rows per core
T = NUM_TREES // TSHARD        # 32 trees per core
KT = D_IN // 128               # 4 contraction tiles
NT = N_INTERNAL * T            # 2016 (node,tree) pairs
LT = N_LEAVES * T              # 2048 (leaf,tree) pairs
NBT = BS // 128                # 8 batch tiles per core
NPT = LT // 128                # 16 contraction chunks for phase D


def _bitrev(x: int, bits: int) -> int:
    r = 0
    for _ in range(bits):
        r = (r << 1) | (x & 1)
        x >>= 1
    return r


# block-recursion orderings (see phase B)
_NODES_PERM = np.array(
    [(2**lvl - 1) + _bitrev(j, lvl) for lvl in range(MAX_DEPTH) for j in range(2**lvl)]
)
_LEAF_PERM = np.array([_bitrev(j, MAX_DEPTH) for j in range(N_LEAVES)])

_BUILT = {}


def _build(use_bias: bool):
    """Build + finalize the per-core Bass program."""
    import concourse.bacc as bacc
    import concourse.tile as tile
    from concourse import mybir
    from concourse.masks import make_identity

    f32 = mybir.dt.float32
    bf16 = mybir.dt.bfloat16
    AF = mybir.ActivationFunctionType
    MUL = mybir.AluOpType.mult
    ADD = mybir.AluOpType.add
    SUB = mybir.AluOpType.subtract

    nc = bacc.Bacc("TRN2", target_bir_lowering=False, debug=False)

    xT = nc.dram_tensor("xT", [KT, 128, BS], bf16, kind="ExternalInput")
    Wf = nc.dram_tensor("Wf", [KT, 128, NT], bf16, kind="ExternalInput")
    # leaf tables host-transposed to [d, (leaf, tree)] for contiguous DMA.
    # gates carry ALL 64 trees (softmax normalizes across the full tree
    # axis), host-rolled so this core's 32 trees sit at t=0:32.
    gt = nc.dram_tensor("gt", [LEAF_DIMS, N_LEAVES * NUM_TREES], bf16,
                        kind="ExternalInput")
    lwt = nc.dram_tensor("lwt", [LEAF_DIMS, LT], bf16, kind="ExternalInput")
    if use_bias:
        bias = nc.dram_tensor("bias", [1, NT], bf16, kind="ExternalInput")
    outT = nc.dram_tensor("outT", [LEAF_DIMS, BS], f32, kind="ExternalOutput")

    # phase A chunking: per btile two [128,1024] PSUM tiles, each covering
    # two single-bank matmul groups
    HALves = [
        (0, [(0, 512), (512, 1024)]),
        (1024, [(1024, 1536), (1536, NT)]),
    ]

    with tile.TileContext(nc) as tc:
        with tc.tile_pool(name="const", bufs=1) as cpool, \
             tc.tile_pool(name="wts", bufs=1) as wpool, \
             tc.tile_pool(name="psA", bufs=3, space="PSUM") as psA, \
             tc.tile_pool(name="psO", bufs=1, space="PSUM") as psO:

            # ---- input DMAs.  x + leaf tables on the ACT HW-DGE ring, the
            # weight matrix on the SP ring, so the early pieces overlap. ----
            xk = wpool.tile([128, KT, BS], bf16, tag="xk")
            nc.scalar.dma_start(
                xk[:, :, 0:256], xT[:, :, 0:256].rearrange("k p b -> p k b"))
            gtile = wpool.tile([128, N_LEAVES, NUM_TREES], bf16, tag="gtile")
            nc.scalar.dma_start(
                gtile[:], gt[:, :].rearrange("d (l t) -> d l t", t=NUM_TREES))
            nc.scalar.dma_start(
                xk[:, :, 256:BS], xT[:, :, 256:BS].rearrange("k p b -> p k b"))
            lwtile = wpool.tile([128, N_LEAVES, T], bf16, tag="lwtile")
            nc.scalar.dma_start(
                lwtile[:], lwt[:, :].rearrange("d (l t) -> d l t", t=T))

            wk = wpool.tile([128, KT, NT], bf16, tag="wk")
            for k in range(KT):
                nc.sync.dma_start(wk[:, k, 0:512], Wf[k, :, 0:512])
            for k in range(KT):
                nc.sync.dma_start(wk[:, k, 512:1024], Wf[k, :, 512:1024])
            for k in range(KT):
                nc.sync.dma_start(wk[:, k, 1024:NT], Wf[k, :, 1024:NT])
            if use_bias:
                bias_sb = cpool.tile([1, NT], bf16, tag="bias")
                nc.sync.dma_start(bias_sb[:], bias[:, :])
                ones1 = cpool.tile([1, 128], bf16, tag="ones1")
                nc.gpsimd.memset(ones1[:], 1.0)

            # ---- ACT queue head: exp of the gate table (its own act-func
            # table; loaded once, before the sigmoid table) ----
            nc.scalar.activation(gtile[:, 0:32, :], gtile[:, 0:32, :], AF.Exp)
            nc.scalar.activation(gtile[:, 32:64, :], gtile[:, 32:64, :], AF.Exp)

            with tc.tile_pool(name="gp", bufs=1) as gpool, \
                 tc.tile_pool(name="pp", bufs=2) as ppool, \
                 tc.tile_pool(name="pfp", bufs=2) as pfpool, \
                 tc.tile_pool(name="ptp", bufs=1) as pTpool, \
                 tc.tile_pool(name="outp", bufs=1) as outpool:

                out_ps = [psO.tile([LEAF_DIMS, 512], f32, tag=f"out_ps{h}",
                                   name=f"out_ps{h}")
                          for h in range(2)]
                out_sb = outpool.tile([LEAF_DIMS, BS], f32, tag="out_sb")

                # PE warm-up: dummy transposes release the HAM clock gate
                # while the first weight DMAs are in flight
                ident32 = cpool.tile([128, 128], f32, tag="ident32")
                make_identity(nc, ident32[:])
                for _wi in range(10):
                    nc.tensor.transpose(out_ps[0][:, 0:128], ident32[:], ident32[:])

                pT_all = pTpool.tile([128, NPT, BS], bf16, tag="pT")
                wT_all = wpool.tile([128, NPT, 128], bf16, tag="wT")
                wsm = wpool.tile([128, N_LEAVES, T], bf16, tag="wsm")
                g_all = gpool.tile([128, NBT, NT], bf16, tag="g")
                # root-level [g | 1-g], written by ACT (sigmoid of +/-z)
                pa_all = gpool.tile([128, NBT, 2 * T], bf16, tag="pa_all")

                def emit_phase0_reduce():
                    # softmax denominator over ALL 64 trees: halving adds
                    # (TensorTensor, 2x), then reciprocal
                    s32 = cpool.tile([128, N_LEAVES, 32], bf16, tag="s32")
                    nc.vector.tensor_add(s32[:], gtile[:, :, 0:32],
                                         gtile[:, :, 32:64])
                    s16 = cpool.tile([128, N_LEAVES, 16], bf16, tag="s16")
                    nc.vector.tensor_add(s16[:], s32[:, :, 0:16], s32[:, :, 16:32])
                    s8 = cpool.tile([128, N_LEAVES, 8], bf16, tag="s8")
                    nc.vector.tensor_add(s8[:], s16[:, :, 0:8], s16[:, :, 8:16])
                    s4 = cpool.tile([128, N_LEAVES, 4], bf16, tag="s4")
                    nc.vector.tensor_add(s4[:], s8[:, :, 0:4], s8[:, :, 4:8])
                    s2 = cpool.tile([128, N_LEAVES, 2], bf16, tag="s2")
                    nc.vector.tensor_add(s2[:], s4[:, :, 0:2], s4[:, :, 2:4])
                    s1 = cpool.tile([128, N_LEAVES], f32, tag="s1")
                    nc.vector.tensor_add(s1[:], s2[:, :, 0], s2[:, :, 1])
                    r = cpool.tile([128, N_LEAVES], bf16, tag="r")
                    with nc.allow_low_precision(reason="softmax recip in bf16"):
                        nc.vector.reciprocal(r[:], s1[:])
                    return r

                def emit_phase0_normalize(r):
                    # w = (e * leaf_weight) * r[l] with r broadcast over t:
                    # two TensorTensor ops instead of 64 per-leaf scalar ops
                    wu = wpool.tile([128, N_LEAVES, T], bf16, tag="wu")
                    nc.vector.tensor_mul(wu[:], gtile[:, :, 0:T], lwtile[:])
                    rb = r[:, :].unsqueeze(2).broadcast_to([128, N_LEAVES, T])
                    nc.vector.tensor_mul(wsm[:], wu[:], rb)
                    # w transposed via DMA xbar on the (now idle) SP ring
                    nc.sync.dma_start_transpose(wT_all[:], wsm[:])

                def emit_phaseD(i):
                    bsl = slice(i * 128, (i + 1) * 128)
                    ps = out_ps[i // 4]
                    osl = slice((i % 4) * 128, (i % 4 + 1) * 128)
                    for j in range(NPT):
                        nc.tensor.matmul(ps[:, osl],
                                         wT_all[:, j, :],
                                         pT_all[:, j, bsl],
                                         start=(j == 0), stop=(j == NPT - 1))
                    # PSUM -> SBUF on ACT (DVE is the busier engine), then DMA
                    nc.scalar.copy(out_sb[:, bsl], ps[:, osl])
                    nc.sync.dma_start(outT[:, bsl], out_sb[:, bsl])

                def emit_phaseB(i0, n):
                    # level doubling for btiles [i0, i0+n) (TensorTensor, 2x;
                    # grouping amortizes the per-op access latency)
                    g = g_all[:, i0:i0 + n, :]
                    pa = ppool.tile([128, 2, 1024], bf16, tag="pa", name="pa")[:, 0:n, :]
                    pb = ppool.tile([128, 2, 1024], bf16, tag="pb", name="pb")[:, 0:n, :]
                    pf = pfpool.tile([128, 2, LT], bf16, tag="pf", name="pf")[:, 0:n, :]
                    cur = pa_all[:, i0:i0 + n, :]
                    for lvl in range(1, MAX_DEPTH):
                        h = (2 ** lvl) * T
                        off = (2 ** lvl - 1) * T
                        dst = pf if lvl == MAX_DEPTH - 1 else \
                            (pb if lvl % 2 else pa)
                        nc.vector.tensor_mul(
                            dst[:, :, 0:h], cur[:, :, 0:h], g[:, :, off:off + h])
                        nc.vector.tensor_sub(
                            dst[:, :, h:2 * h], cur[:, :, 0:h], dst[:, :, 0:h])
                        cur = dst
                    for q in range(n):
                        nc.sync.dma_start_transpose(
                            pT_all[:, :, (i0 + q) * 128:(i0 + q + 1) * 128],
                            pf[:, q, :])

                for i in range(NBT):
                    bsl = slice(i * 128, (i + 1) * 128)
                    # ---- phase A ----
                    for off, chunks in HALves:
                        w_tot = chunks[-1][1] - off
                        lg = psA.tile([128, 1024], f32, tag="lg")
                        for c0, c1 in chunks:
                            r0, r1 = c0 - off, c1 - off
                            for k in range(KT):
                                nc.tensor.matmul(
                                    lg[:, r0:r1], xk[:, k, bsl], wk[:, k, c0:c1],
                                    start=(k == 0),
                                    stop=(k == KT - 1 and not use_bias),
                                )
                            if use_bias:
                                nc.tensor.matmul(
                                    lg[:, r0:r1], ones1[:], bias_sb[:, c0:c1],
                                    start=False, stop=True,
                                )
                        nc.scalar.activation(
                            g_all[:, i, off:off + w_tot], lg[:, 0:w_tot],
                            AF.Sigmoid)
                        if off == 0:
                            # root probabilities straight from the logits:
                            # sigmoid(z), sigmoid(-z) -> [g0 | 1-g0]
                            nc.scalar.activation(
                                pa_all[:, i, 0:T], lg[:, 0:T], AF.Sigmoid)
                            nc.scalar.activation(
                                pa_all[:, i, T:2 * T], lg[:, 0:T], AF.Sigmoid,
                                scale=-1.0)

                    if i == 0:
                        emit_phaseB(0, 1)
                        r = emit_phase0_reduce()
                    if i == 1:
                        emit_phaseB(1, 1)
                        emit_phase0_normalize(r)
                    if i == 3:
                        emit_phaseB(2, 2)
                    if i == 5:
                        emit_phaseB(4, 2)
                    if i == 6:
                        emit_phaseB(6, 1)
                    if i == 7:
                        emit_phaseB(7, 1)
                    # phase D trails phase A so the in-order PE queue never
                    # waits on a transpose while phase-A work is ready
                    if i >= 4:
                        emit_phaseD(i - 4)
                for i in range(NBT - 4, NBT):
                    emit_phaseD(i)

    nc.finalize()
    return nc


def _get_nc(use_bias: bool):
    if use_bias not in _BUILT:
        _BUILT[use_bias] = _build(use_bias)
    return _BUILT[use_bias]


def _make_in_maps(x, W, b, leaf_weight, gates):
    x = np.ascontiguousarray(np.asarray(x, dtype=np.float32))
    W = np.asarray(W, dtype=np.float32)
    b = np.asarray(b, dtype=np.float32)
    leaf_weight = np.asarray(leaf_weight, dtype=np.float32)
    gates = np.asarray(gates, dtype=np.float32)

    use_bias = bool(np.any(b))
    Wp = W[_NODES_PERM]                                   # [63, 512, 64]
    gp = gates[_LEAF_PERM]                                # [64, 128, 64]
    lp = leaf_weight[_LEAF_PERM]

    in_maps = []
    for c in range(N_CORES):
        bs = c % BSHARD
        th = c // BSHARD
        ts = slice(th * T, (th + 1) * T)
        troll = np.roll(np.arange(NUM_TREES), -th * T)    # own half first
        xs = x[bs * BS:(bs + 1) * BS]                     # [1024, 512]
        xTc = np.ascontiguousarray(xs.T.reshape(KT, 128, BS).astype(BF16))
        Wfc = np.ascontiguousarray(
            Wp[:, :, ts].transpose(1, 0, 2).reshape(KT, 128, NT).astype(BF16))
        gtc = np.ascontiguousarray(
            gp[:, :, troll].transpose(1, 0, 2).reshape(LEAF_DIMS, -1).astype(BF16))
        lwc = np.ascontiguousarray(
            lp[:, :, ts].transpose(1, 0, 2).reshape(LEAF_DIMS, LT).astype(BF16))
        m = {"xT": xTc, "Wf": Wfc, "gt": gtc, "lwt": lwc}
        if use_bias:
            m["bias"] = np.ascontiguousarray(
                b[_NODES_PERM][:, ts].reshape(1, NT).astype(BF16))
        in_maps.append(m)
    return use_bias, in_maps


def kernel(x, W, b, leaf_weight, gates):
    from concourse.bass_utils import run_bass_kernel_spmd

    use_bias, in_maps = _make_in_maps(x, W, b, leaf_weight, gates)
    nc = _get_nc(use_bias)

    res = run_bass_kernel_spmd(nc, in_maps, core_ids=list(range(N_CORES)))
    out = np.empty((BATCH, LEAF_DIMS), dtype=np.float32)
    for c in range(BSHARD):
        out[c * BS:(c + 1) * BS] = (
            res.results[c]["outT"] + res.results[c + BSHARD]["outT"]).T
    return out
